# revision 11
# baseline (speedup 1.0000x reference)
"""CLIP-style contrastive (NT-Xent) loss on 8 Trainium2 NeuronCores.

v2 strategy (data-parallel, per sharding hint):
  - Shard the batch (4096) across 8 cores: 512 rows of x_image/x_text each.
  - Encoder (1024->512) runs in fp8-e4m3 DoubleRow matmuls (0.5 cyc/row,
    contraction packed as [128 partitions x 2 k-planes]); projection head
    in bf16.  Activations stay transposed [feat, batch] so stored weights
    are the stationary lhsT directly.
  - L2-normalize via a table-free DVE rsqrt (magic-seed Newton), keeping
    the ScalarE activation table pinned to Exp for the whole kernel (one
    table load at t=0 instead of 4 Ln/Exp reloads).
  - Projections cast to fp8, AllGather'd per modality (64KB payload) as
    soon as each tower finishes; input DMAs are split across the SP and
    Pool HWDGE queues so the two towers load in parallel.
  - Each core computes its 1024 rows of the global 8192x8192 similarity
    in fp8 DoubleRow matmuls (2-plane operands, plane 1 zeroed, so PSUM
    holds 2*sim).  Row sums of exp(sim/t) are split across two engines:
    ScalarE does exp via activation(accum_out) on the units that contain
    the diagonal (so the host's exp(diag/t) subtraction matches the
    device's table-exp), the Vector engine does the rest via a
    Schraudolph bit-trick: int16(2sim*a1+a2) bit-cast to bf16 is
    exp(sim/t) to ~1.5%, then a 2-byte tensor_reduce accumulates rows.
  - Device returns per 128-row-chunk: per-unit partial sums T, plus
    pos_r (fp32 z1.z2) and diag_r (|z_fp8|^2, matching the sim matmul's
    self-column exactly).  Host finishes in fp64:
        T'_r   = T_r - exp(diag_r/t) + exp(pos_r/t)
        loss_r = log(T'_r) - pos_r/t
"""

import os

os.environ.setdefault("NEURON_RT_DBG_RDH_CC", "0")

import numpy as np
import ml_dtypes

import concourse.bacc as bacc
import concourse.bass as bass
import concourse.mybir as mybir
import concourse.tile as tile
from concourse.bass_utils import run_bass_kernel_spmd

NCORES = 8
B, DIN, DE, DH, DP = 4096, 1024, 512, 256, 128
S = B // NCORES            # 512: per-core batch shard
ROWS = 2 * S               # 1024 sim rows owned per core
N = 2 * B                  # 8192 global rows
TEMP = 0.07
INV_T = 1.0 / TEMP

F32 = mybir.dt.float32
F32R = mybir.dt.float32r
BF16 = mybir.dt.bfloat16
F8 = mybir.dt.float8e4
I16 = mybir.dt.int16
I32 = mybir.dt.int32
NP_F8 = ml_dtypes.float8_e4m3
NP_BF16 = ml_dtypes.bfloat16
DR = mybir.MatmulPerfMode.DoubleRow

# Schraudolph bf16 exp constants.  PSUM sim values are doubled (2 planes
# both carrying data would double; here plane1=0 so psum = 2*sim only in
# spirit -- see ZERO_PLANE note below).  psum p = 2*sim is NOT used; with a
# zero plane p = sim, so a1 uses full INV_T.
LOG2E = 1.4426950408889634
A1 = float(np.float32(128.0 * LOG2E * INV_T))   # 2638.0709...
A2 = float(np.float32(16256.0 - 7.25))          # 127*128 + C, C tuned offline

NRC = ROWS // 128          # 8 row chunks
UW = 2048                  # columns per sim unit (4 PSUM banks)
NU = N // UW               # 4 units per row chunk (2 img + 2 txt)
# per-rc extra ACT units (rebalance ACT vs DVE): txt units for rc<4
EXTRA_ACT = 2

OUT_COLS = 20              # T(8) | pos(4) | diag_img(4) | diag_txt(4)

RSQRT_MAGIC = 0x5F3759DF

_CACHE: dict = {}


def _build():
    nc = bacc.Bacc("TRN2", target_bir_lowering=False, debug=False,
                   num_devices=NCORES)

    t_in = {}
    for m in ("img", "txt"):
        # fp8 DoubleRow layouts, host-prepped:
        #   xT8 [128, 8, 512]: [p, ko, b] = x[b, 128*ko + p]
        #   We8 [128, 8, 512]: [p, ko, m] = We[128*ko + p, m]
        t_in[f"xT8_{m}"] = nc.dram_tensor(f"xT8_{m}", [128, (DIN // 128) * S],
                                          F8, kind="ExternalInput")
        t_in[f"We8_{m}"] = nc.dram_tensor(f"We8_{m}", [128, (DIN // 128) * DE],
                                          F8, kind="ExternalInput")
        t_in[f"Wp1_{m}"] = nc.dram_tensor(f"Wp1_{m}", [DE, DH], BF16,
                                          kind="ExternalInput")
        t_in[f"Wp2_{m}"] = nc.dram_tensor(f"Wp2_{m}", [DH, DP], BF16,
                                          kind="ExternalInput")
        t_in[f"beT_{m}"] = nc.dram_tensor(f"beT_{m}", [128, DE // 128], F32,
                                          kind="ExternalInput")
        t_in[f"bp1T_{m}"] = nc.dram_tensor(f"bp1T_{m}", [128, DH // 128], F32,
                                           kind="ExternalInput")
        t_in[f"bp2T_{m}"] = nc.dram_tensor(f"bp2T_{m}", [128, DP // 128], F32,
                                           kind="ExternalInput")
    out_t = nc.dram_tensor("parts", [128, OUT_COLS], F32,
                           kind="ExternalOutput")

    with tile.TileContext(nc) as tc:
        _emit(nc, tc, t_in, out_t)
    nc.compile()
    return nc


def _load_tower(nc, wpool, t_in, m, q):
    """DMA one tower's operands on queue engine `q` in consumption order."""
    KO = DIN // 128
    x8 = wpool.tile([128, KO * S], F8, name=f"x8_{m}")
    w8 = wpool.tile([128, KO * DE], F8, name=f"w8_{m}")
    # halves: first matmul group needs all ko for mm=0; interleave x/w halves
    for half in range(2):
        sl = slice(half * (KO // 2) * DE, (half + 1) * (KO // 2) * DE)
        q.dma_start(out=w8[:, sl], in_=t_in[f"We8_{m}"][:, sl])
        slx = slice(half * (KO // 2) * S, (half + 1) * (KO // 2) * S)
        q.dma_start(out=x8[:, slx], in_=t_in[f"xT8_{m}"][:, slx])
    wp1 = wpool.tile([128, (DE // 128) * DH], BF16, name=f"wp1_{m}")
    for k in range(DE // 128):
        q.dma_start(out=wp1[:, k * DH:(k + 1) * DH],
                    in_=t_in[f"Wp1_{m}"][128 * k:128 * (k + 1), :])
    wp2 = wpool.tile([128, (DH // 128) * DP], BF16, name=f"wp2_{m}")
    for k in range(DH // 128):
        q.dma_start(out=wp2[:, k * DP:(k + 1) * DP],
                    in_=t_in[f"Wp2_{m}"][128 * k:128 * (k + 1), :])
    beT = wpool.tile([128, DE // 128], F32, name=f"beT_{m}")
    q.dma_start(out=beT[:], in_=t_in[f"beT_{m}"][:, :])
    bp1T = wpool.tile([128, DH // 128], F32, name=f"bp1T_{m}")
    q.dma_start(out=bp1T[:], in_=t_in[f"bp1T_{m}"][:, :])
    bp2T = wpool.tile([128, DP // 128], F32, name=f"bp2T_{m}")
    q.dma_start(out=bp2T[:], in_=t_in[f"bp2T_{m}"][:, :])
    return dict(x8=x8, w8=w8, wp1=wp1, wp2=wp2, beT=beT, bp1T=bp1T, bp2T=bp2T)


def _rsqrt_dve(nc, psb, ss_ps, tag):
    """inv = rsqrt(ss) on DVE, table-free: magic seed + 2 Newton iters.

    ss_ps: [1, S] fp32 PSUM.  Returns [1, S] fp32 SBUF."""
    mul = mybir.AluOpType.mult
    add = mybir.AluOpType.add
    shr = mybir.AluOpType.logical_shift_right
    ss = psb.tile([1, S], F32, tag=f"{tag}_ss", name=f"{tag}_ss")
    nc.vector.tensor_copy(ss[:], ss_ps[:])
    u = psb.tile([1, S], I32, tag=f"{tag}_u", name=f"{tag}_u")
    nc.vector.tensor_scalar(out=u[:], in0=ss[:].bitcast(I32),
                            scalar1=1, scalar2=None, op0=shr)
    y0 = psb.tile([1, S], I32, tag=f"{tag}_y0", name=f"{tag}_y0")
    nc.vector.tensor_scalar(out=y0[:], in0=u[:], scalar1=-1,
                            scalar2=RSQRT_MAGIC, op0=mul, op1=add)
    y = y0[:].bitcast(F32)
    for it in range(2):
        t1 = psb.tile([1, S], F32, tag=f"{tag}_t1{it}", name=f"{tag}_t1{it}")
        nc.vector.tensor_mul(t1[:], y, y)                     # y^2
        t2 = psb.tile([1, S], F32, tag=f"{tag}_t2{it}", name=f"{tag}_t2{it}")
        # t2 = (y^2 * -0.5) * ss  (scalar_tensor_tensor)
        nc.vector.scalar_tensor_tensor(out=t2[:], in0=t1[:], scalar=-0.5,
                                       in1=ss[:], op0=mul, op1=mul)
        t3 = psb.tile([1, S], BF16 if it == 1 else F32,
                      tag=f"{tag}_t3{it}", name=f"{tag}_t3{it}")
        # y = y * (t2 + 1.5)
        nc.vector.scalar_tensor_tensor(out=t3[:], in0=t2[:], scalar=1.5,
                                       in1=y, op0=add, op1=mul)
        y = t3[:]
    return y


def _tower(nc, tc, pools, w, m, consts):
    """Project one tower and normalize.  Returns (zn fp32 [128,S],
    znb8 fp8 [128,2,S] with plane1 zeroed elsewhere)."""
    pps, psb, apool = pools
    add = mybir.AluOpType.add
    mx = mybir.AluOpType.max
    ones_col, ones_colb, ones_rowb = consts
    KO = DIN // 128

    h = psb.tile([128, (DE // 128) * S], BF16, tag="h")
    for mm in range(DE // 128):
        ph = pps.tile([128, S], F32, tag="ps")
        for k2 in range(0, KO, 2):
            nc.tensor.matmul(
                ph[:],
                w["w8"][:, :].rearrange("p (ko m) -> p ko m", ko=KO)
                    [:, k2:k2 + 2, 128 * mm:128 * (mm + 1)],
                w["x8"][:, :].rearrange("p (ko b) -> p ko b", ko=KO)
                    [:, k2:k2 + 2, :],
                start=(k2 == 0), stop=(k2 == KO - 2), perf_mode=DR)
        nc.scalar.activation(
            h[:, mm * S:(mm + 1) * S], ph[:],
            mybir.ActivationFunctionType.Identity,
            bias=w["beT"][:, mm:mm + 1], scale=1.0)
    g = psb.tile([128, (DH // 128) * S], BF16, tag="g")
    for mm in range(DH // 128):
        pg = pps.tile([128, S], F32, tag="ps")
        for k in range(DE // 128):
            nc.tensor.matmul(
                pg[:],
                w["wp1"][:, k * DH + 128 * mm: k * DH + 128 * (mm + 1)],
                h[:, k * S:(k + 1) * S],
                start=(k == 0), stop=(k == DE // 128 - 1))
        nc.scalar.activation(
            g[:, mm * S:(mm + 1) * S], pg[:],
            mybir.ActivationFunctionType.Relu,
            bias=w["bp1T"][:, mm:mm + 1], scale=1.0)
    pz = pps.tile([128, S], F32, tag="ps")
    for k in range(DH // 128):
        nc.tensor.matmul(pz[:], w["wp2"][:, k * DP: k * DP + 128],
                         g[:, k * S:(k + 1) * S],
                         start=(k == 0), stop=(k == DH // 128 - 1))
    z = psb.tile([128, S], F32, tag=f"z_{m}")
    nc.scalar.activation(z[:], pz[:], mybir.ActivationFunctionType.Identity,
                         bias=w["bp2T"][:, 0:1], scale=1.0)

    # ss = col sums of z^2 via bf16 ones-matmul (1 cyc/row)
    sq = psb.tile([128, S], BF16, tag="sq")
    nc.vector.tensor_mul(sq[:], z[:], z[:])
    pssq = pps.tile([1, S], F32, tag="ps")
    nc.tensor.matmul(pssq[:], ones_colb[:], sq[:], start=True, stop=True)
    inv = _rsqrt_dve(nc, psb, pssq, f"rs_{m}")
    pinvb = pps.tile([128, S], F32, tag="ps")
    nc.tensor.matmul(pinvb[:], ones_rowb[:], inv, start=True, stop=True)
    zn = apool.tile([128, S], F32, name=f"zn_{m}")
    nc.vector.tensor_mul(zn[:], z[:], pinvb[:])
    znb8 = apool.tile([128, 2, S], F8, name=f"znb8_{m}")
    nc.vector.tensor_copy(znb8[:, 0, :], zn[:])
    return zn, znb8


def _emit(nc, tc, t_in, out_t):
    Exp = mybir.ActivationFunctionType.Exp
    add = mybir.AluOpType.add
    mul = mybir.AluOpType.mult

    with tc.tile_pool(name="const", bufs=1) as cpool, \
         tc.tile_pool(name="wpool", bufs=1) as wpool, \
         tc.tile_pool(name="actpool", bufs=1) as apool, \
         tc.tile_pool(name="projsb", bufs=2) as psb, \
         tc.tile_pool(name="psum", bufs=2, space="PSUM") as pps, \
         tc.tile_pool(name="escp", bufs=2) as escp, \
         tc.tile_pool(name="dram", bufs=1, space="DRAM") as dram:

        ones_col = cpool.tile([128, 1], F32)
        nc.vector.memset(ones_col[:], 1.0)
        ones_colb = cpool.tile([128, 1], BF16)
        nc.vector.memset(ones_colb[:], 1.0)
        ones_rowb = cpool.tile([1, 128], BF16)
        nc.vector.memset(ones_rowb[:], 1.0)
        # Pin the ScalarE table to Exp once, at t=0.
        dummy = cpool.tile([1, 1], F32)
        nc.scalar.activation(dummy[:], ones_col[0:1, 0:1], Exp)

        # gathered projections [128, 2, 4096]; plane 1 stays zero (DR pad)
        zf8 = {m: apool.tile([128, 2, B], F8, name=f"zf8_{m}")
               for m in ("img", "txt")}
        for m in ("img", "txt"):
            nc.gpsimd.memset(zf8[m][:, 1, :], 0.0)

        # tower input DMAs: img on SP queue, txt on Pool queue (parallel)
        w_all = {"img": _load_tower(nc, wpool, t_in, "img", nc.sync),
                 "txt": _load_tower(nc, wpool, t_in, "txt", nc.gpsimd)}

        zn, znb8, cc_out = {}, {}, {}
        for m in ("img", "txt"):
            zn[m], znb8[m] = _tower(nc, tc, (pps, psb, apool), w_all[m], m,
                                    (ones_col, ones_colb, ones_rowb))
            nc.gpsimd.memset(znb8[m][:, 1, :], 0.0)
            # bounce fp8 plane-0 to DRAM on the ScalarE HWDGE queue
            cc_in = dram.tile([128, S], F8, name=f"cc_in_{m}")
            nc.scalar.dma_start(out=cc_in[:, :], in_=znb8[m][:, 0, :])
            cc_o = dram.tile([128 * NCORES, S], F8, name=f"cc_out_{m}",
                             addr_space="Shared")
            nc.gpsimd.collective_compute(
                "AllGather", mybir.AluOpType.bypass,
                replica_groups=[list(range(NCORES))],
                ins=[cc_in[:]], outs=[cc_o[:]])
            cc_out[m] = cc_o
            # gather loads ride the SP queue (idle after tower loads)
            for j in range(NCORES):
                nc.sync.dma_start(
                    out=zf8[m][:, 0, S * j: S * (j + 1)],
                    in_=cc_o[128 * j:128 * (j + 1), :])

        # pos / self-diag rows ([1, S] each) -> [128, 4] via DRAM scatter.
        # diag must be |z_fp8|^2 to cancel the sim matmul's self column.
        zq = {}
        for m in ("img", "txt"):
            zq[m] = psb.tile([128, S], BF16, tag=f"zq_{m}", name=f"zq_{m}")
            nc.vector.tensor_copy(zq[m][:], znb8[m][:, 0, :])
        rows_d = dram.tile([3, S], F32)
        for r, (a, b, src) in enumerate((("img", "txt", zn),
                                         ("img", "img", zq),
                                         ("txt", "txt", zq))):
            prod = psb.tile([128, S], F32, tag="prod")
            if src is zn:
                nc.vector.tensor_mul(prod[:], zn[a][:], zn[b][:])
            else:
                nc.vector.tensor_mul(prod[:], zq[a][:], zq[b][:])
            pr = pps.tile([1, S], F32, tag="ps")
            nc.tensor.matmul(pr[:], ones_col[:], prod[:], start=True,
                             stop=True)
            row_sb = psb.tile([1, S], F32, tag="rowsb")
            nc.vector.tensor_copy(row_sb[:], pr[:])
            nc.scalar.dma_start(out=rows_d[r:r + 1, :], in_=row_sb[:])
        pdT = apool.tile([128, 12], F32)
        for r in range(3):
            nc.scalar.dma_start(
                out=pdT[:, 4 * r:4 * (r + 1)],
                in_=rows_d[r:r + 1, :].rearrange("o (c p) -> (o p) c", p=128))

        # ---- sim units ----
        # unit (rc, uu): rows 128rc..128rc+128, cols [uu*2048, (uu+1)*2048)
        # uu 0..1 = img cols, 2..3 = txt cols.
        # ACT units: img units for rc<4, txt units for rc>=4 (diag lives
        # there for every core), + EXTRA_ACT txt units of rc<4 appended.
        # DVE units: the rest (Schraudolph).
        stats = apool.tile([128, NRC * NU], F32)

        def emit_unit(rc, uu, eng):
            m = "img" if uu < 2 else "txt"
            coff = (uu % 2) * UW
            lsrc = znb8["img"] if rc < 4 else znb8["txt"]
            lhs = lsrc[:, :, 128 * (rc % 4):128 * (rc % 4 + 1)]
            ps = pps.tile([128, UW], F32, tag="ps")
            for q in range(UW // 512):
                nc.tensor.matmul(
                    ps[:, 512 * q:512 * (q + 1)], lhs,
                    zf8[m][:, :, coff + 512 * q: coff + 512 * (q + 1)],
                    start=True, stop=True, perf_mode=DR)
            sc = stats[:, NU * rc + uu: NU * rc + uu + 1]
            if eng == "A":
                esc = escp.tile([128, UW], BF16, tag="esc")
                nc.scalar.activation(esc[:], ps[:], Exp, scale=INV_T,
                                     accum_out=sc)
            else:
                t16 = escp.tile([128, UW], I16, tag="t16")
                nc.vector.tensor_scalar(out=t16[:], in0=ps[:],
                                        scalar1=A1, scalar2=A2,
                                        op0=mul, op1=add)
                # accumulate rows via a bf16 identity pass (2-byte DVE fast
                # mode) instead of tensor_reduce (1 elem/cyc); every 3rd
                # second-pass rides the Pool engine.
                d2 = escp.tile([128, UW], BF16, tag="d2")
                eng2 = nc.gpsimd if eng == "P" else nc.vector
                eng2.tensor_scalar(out=d2[:], in0=t16[:].bitcast(BF16),
                                   scalar1=1.0, scalar2=0.0, op0=mul,
                                   op1=add, accum_out=sc)

        units = []
        # img phase: ACT rc0-3, DVE rc4-7, interleaved for PSUM rotation
        for rc in range(4):
            for uu in range(2):
                units.append((rc, uu, "A"))
                units.append((rc + 4, uu, "D"))
        # txt phase: ACT rc4-7 (+extras from rc0-3), DVE rest of rc0-3
        extras = [(rc, uu) for rc in range(4) for uu in (2, 3)][:EXTRA_ACT]
        for rc in range(4):
            for uu in (2, 3):
                units.append((rc + 4, uu, "A"))
                units.append((rc, uu, "A" if (rc, uu) in extras else "D"))
        for rc, uu, eng in units:
            emit_unit(rc, uu, eng)

        # ---- gather outputs: T (8) | pos(4) | diag1(4) | diag2(4) ----
        outv = apool.tile([128, OUT_COLS], F32)
        nc.vector.tensor_reduce(
            out=outv[:, 0:NRC],
            in_=stats[:].rearrange("p (r u) -> p r u", u=NU),
            axis=mybir.AxisListType.X, op=add)
        nc.vector.tensor_copy(outv[:, NRC:NRC + 12], pdT[:])
        nc.sync.dma_start(out=out_t[:, :], in_=outv[:])


def _prep_in_maps(inputs):
    host = {}
    KO = DIN // 128
    for m in ("img", "txt"):
        We = np.asarray(inputs[f"We_{m}"], np.float32)     # [1024, 512]
        # [p, ko, m] = We[128*ko + p, m]
        host[f"We8_{m}"] = np.ascontiguousarray(
            We.reshape(KO, 128, DE).transpose(1, 0, 2).reshape(128, KO * DE)
        ).astype(NP_F8)
        host[f"Wp1_{m}"] = np.ascontiguousarray(inputs[f"Wp1_{m}"]).astype(NP_BF16)
        host[f"Wp2_{m}"] = np.ascontiguousarray(inputs[f"Wp2_{m}"]).astype(NP_BF16)
        host[f"beT_{m}"] = np.ascontiguousarray(
            np.asarray(inputs[f"be_{m}"], np.float32).reshape(DE // 128, 128).T)
        host[f"bp1T_{m}"] = np.ascontiguousarray(
            np.asarray(inputs[f"bp1_{m}"], np.float32).reshape(DH // 128, 128).T)
        host[f"bp2T_{m}"] = np.ascontiguousarray(
            np.asarray(inputs[f"bp2_{m}"], np.float32).reshape(DP // 128, 128).T)
    x = {"img": np.asarray(inputs["x_image"], np.float32),
         "txt": np.asarray(inputs["x_text"], np.float32)}
    in_maps = []
    for c in range(NCORES):
        mp = dict(host)
        for m in ("img", "txt"):
            xs = x[m][c * S:(c + 1) * S]                    # [512, 1024]
            # [p, ko, b] = x[b, 128*ko + p]
            mp[f"xT8_{m}"] = np.ascontiguousarray(
                xs.T.reshape(KO, 128, S).transpose(1, 0, 2).reshape(128, KO * S)
            ).astype(NP_F8)
        in_maps.append(mp)
    return in_maps


def _finish_host(results):
    """Host-side fp64 finish: combine per-core T/pos/diag into the loss."""
    total = 0.0
    t = TEMP
    for c in range(NCORES):
        p = np.asarray(results[c]["parts"], np.float64)
        T = p[:, 0:8]           # [128, rc]
        pos = p[:, 8:12]        # [128, k]  (k = batch chunk within shard)
        d1 = p[:, 12:16]
        d2 = p[:, 16:20]
        for rc in range(8):
            k = rc % 4
            dg = d1[:, k] if rc < 4 else d2[:, k]
            Tp = T[:, rc] - np.exp(dg / t) + np.exp(pos[:, k] / t)
            total += float(np.sum(np.log(Tp) - pos[:, k] / t))
    return np.float32(total / N)


def kernel(**inputs) -> np.ndarray:
    nc = _CACHE.get("nc")
    if nc is None:
        nc = _build()
        _CACHE["nc"] = nc
    res = run_bass_kernel_spmd(nc, _prep_in_maps(inputs),
                               core_ids=list(range(NCORES)))
    return _finish_host(res.results)


# revision 12
# speedup vs baseline: 1.0606x; 1.0606x over previous
"""CLIP-style contrastive (NT-Xent) loss on 8 Trainium2 NeuronCores.

v2 strategy (data-parallel, per sharding hint):
  - Shard the batch (4096) across 8 cores: 512 rows of x_image/x_text each.
  - Encoder (1024->512) runs in fp8-e4m3 DoubleRow matmuls (0.5 cyc/row,
    contraction packed as [128 partitions x 2 k-planes]); projection head
    in bf16.  Activations stay transposed [feat, batch] so stored weights
    are the stationary lhsT directly.
  - L2-normalize via a table-free DVE rsqrt (magic-seed Newton), keeping
    the ScalarE activation table pinned to Exp for the whole kernel (one
    table load at t=0 instead of 4 Ln/Exp reloads).
  - Projections cast to fp8, AllGather'd per modality (64KB payload) as
    soon as each tower finishes; input DMAs are split across the SP and
    Pool HWDGE queues so the two towers load in parallel.
  - Each core computes its 1024 rows of the global 8192x8192 similarity
    in fp8 DoubleRow matmuls (2-plane operands, plane 1 zeroed, so PSUM
    holds 2*sim).  Row sums of exp(sim/t) are split across two engines:
    ScalarE does exp via activation(accum_out) on the units that contain
    the diagonal (so the host's exp(diag/t) subtraction matches the
    device's table-exp), the Vector engine does the rest via a
    Schraudolph bit-trick: int16(2sim*a1+a2) bit-cast to bf16 is
    exp(sim/t) to ~1.5%, then a 2-byte tensor_reduce accumulates rows.
  - Device returns per 128-row-chunk: per-unit partial sums T, plus
    pos_r (fp32 z1.z2) and diag_r (|z_fp8|^2, matching the sim matmul's
    self-column exactly).  Host finishes in fp64:
        T'_r   = T_r - exp(diag_r/t) + exp(pos_r/t)
        loss_r = log(T'_r) - pos_r/t
"""

import os

os.environ.setdefault("NEURON_RT_DBG_RDH_CC", "0")

import numpy as np
import ml_dtypes

import concourse.bacc as bacc
import concourse.bass as bass
import concourse.mybir as mybir
import concourse.tile as tile
from concourse.bass_utils import run_bass_kernel_spmd

NCORES = 8
B, DIN, DE, DH, DP = 4096, 1024, 512, 256, 128
S = B // NCORES            # 512: per-core batch shard
ROWS = 2 * S               # 1024 sim rows owned per core
N = 2 * B                  # 8192 global rows
TEMP = 0.07
INV_T = 1.0 / TEMP

F32 = mybir.dt.float32
F32R = mybir.dt.float32r
BF16 = mybir.dt.bfloat16
F8 = mybir.dt.float8e4
I16 = mybir.dt.int16
I32 = mybir.dt.int32
NP_F8 = ml_dtypes.float8_e4m3
NP_BF16 = ml_dtypes.bfloat16
DR = mybir.MatmulPerfMode.DoubleRow

# Schraudolph bf16 exp constants.  PSUM sim values are doubled (2 planes
# both carrying data would double; here plane1=0 so psum = 2*sim only in
# spirit -- see ZERO_PLANE note below).  psum p = 2*sim is NOT used; with a
# zero plane p = sim, so a1 uses full INV_T.
LOG2E = 1.4426950408889634
A1 = float(np.float32(128.0 * LOG2E * INV_T))   # 2638.0709...
A2 = float(np.float32(16256.0 - 7.25))          # 127*128 + C, C tuned offline

NRC = ROWS // 128          # 8 row chunks
UW = 2048                  # columns per sim unit (4 PSUM banks)
NU = N // UW               # 4 units per row chunk (2 img + 2 txt)
# per-rc extra ACT units (rebalance ACT vs DVE): txt units for rc<4
EXTRA_ACT = 6

OUT_COLS = 20              # T(8) | pos(4) | diag_img(4) | diag_txt(4)

RSQRT_MAGIC = 0x5F3759DF

_CACHE: dict = {}


def _build():
    nc = bacc.Bacc("TRN2", target_bir_lowering=False, debug=False,
                   num_devices=NCORES)

    t_in = {}
    for m in ("img", "txt"):
        # fp8 DoubleRow layouts, host-prepped:
        #   xT8 [128, 8, 512]: [p, ko, b] = x[b, 128*ko + p]
        #   We8 [128, 8, 512]: [p, ko, m] = We[128*ko + p, m]
        t_in[f"xT8_{m}"] = nc.dram_tensor(f"xT8_{m}", [128, (DIN // 128) * S],
                                          F8, kind="ExternalInput")
        t_in[f"We8_{m}"] = nc.dram_tensor(f"We8_{m}", [128, (DIN // 128) * DE],
                                          F8, kind="ExternalInput")
        t_in[f"Wp1_{m}"] = nc.dram_tensor(f"Wp1_{m}", [DE, DH], BF16,
                                          kind="ExternalInput")
        t_in[f"Wp2_{m}"] = nc.dram_tensor(f"Wp2_{m}", [DH, DP], BF16,
                                          kind="ExternalInput")
        t_in[f"beT_{m}"] = nc.dram_tensor(f"beT_{m}", [128, DE // 128], F32,
                                          kind="ExternalInput")
        t_in[f"bp1T_{m}"] = nc.dram_tensor(f"bp1T_{m}", [128, DH // 128], F32,
                                           kind="ExternalInput")
        t_in[f"bp2T_{m}"] = nc.dram_tensor(f"bp2T_{m}", [128, DP // 128], F32,
                                           kind="ExternalInput")
    out_t = nc.dram_tensor("parts", [128, OUT_COLS], F32,
                           kind="ExternalOutput")

    with tile.TileContext(nc) as tc:
        _emit(nc, tc, t_in, out_t)
    nc.compile()
    return nc


def _load_tower(nc, wpool, t_in, m, q):
    """DMA one tower's operands on queue engine `q` in consumption order."""
    KO = DIN // 128
    x8 = wpool.tile([128, KO * S], F8, name=f"x8_{m}")
    w8 = wpool.tile([128, KO * DE], F8, name=f"w8_{m}")
    # halves: first matmul group needs all ko for mm=0; interleave x/w halves
    for half in range(2):
        sl = slice(half * (KO // 2) * DE, (half + 1) * (KO // 2) * DE)
        q.dma_start(out=w8[:, sl], in_=t_in[f"We8_{m}"][:, sl])
        slx = slice(half * (KO // 2) * S, (half + 1) * (KO // 2) * S)
        q.dma_start(out=x8[:, slx], in_=t_in[f"xT8_{m}"][:, slx])
    wp1 = wpool.tile([128, (DE // 128) * DH], BF16, name=f"wp1_{m}")
    for k in range(DE // 128):
        q.dma_start(out=wp1[:, k * DH:(k + 1) * DH],
                    in_=t_in[f"Wp1_{m}"][128 * k:128 * (k + 1), :])
    wp2 = wpool.tile([128, (DH // 128) * DP], BF16, name=f"wp2_{m}")
    for k in range(DH // 128):
        q.dma_start(out=wp2[:, k * DP:(k + 1) * DP],
                    in_=t_in[f"Wp2_{m}"][128 * k:128 * (k + 1), :])
    beT = wpool.tile([128, DE // 128], F32, name=f"beT_{m}")
    q.dma_start(out=beT[:], in_=t_in[f"beT_{m}"][:, :])
    bp1T = wpool.tile([128, DH // 128], F32, name=f"bp1T_{m}")
    q.dma_start(out=bp1T[:], in_=t_in[f"bp1T_{m}"][:, :])
    bp2T = wpool.tile([128, DP // 128], F32, name=f"bp2T_{m}")
    q.dma_start(out=bp2T[:], in_=t_in[f"bp2T_{m}"][:, :])
    return dict(x8=x8, w8=w8, wp1=wp1, wp2=wp2, beT=beT, bp1T=bp1T, bp2T=bp2T)


def _rsqrt_dve(nc, psb, ss_ps, tag):
    """inv = rsqrt(ss) on DVE, table-free: magic seed + 2 Newton iters.

    ss_ps: [1, S] fp32 PSUM.  Returns [1, S] fp32 SBUF."""
    mul = mybir.AluOpType.mult
    add = mybir.AluOpType.add
    shr = mybir.AluOpType.logical_shift_right
    ss = psb.tile([1, S], F32, tag=f"{tag}_ss", name=f"{tag}_ss")
    nc.vector.tensor_copy(ss[:], ss_ps[:])
    u = psb.tile([1, S], I32, tag=f"{tag}_u", name=f"{tag}_u")
    nc.vector.tensor_scalar(out=u[:], in0=ss[:].bitcast(I32),
                            scalar1=1, scalar2=None, op0=shr)
    y0 = psb.tile([1, S], I32, tag=f"{tag}_y0", name=f"{tag}_y0")
    nc.vector.tensor_scalar(out=y0[:], in0=u[:], scalar1=-1,
                            scalar2=RSQRT_MAGIC, op0=mul, op1=add)
    y = y0[:].bitcast(F32)
    for it in range(2):
        t1 = psb.tile([1, S], F32, tag=f"{tag}_t1{it}", name=f"{tag}_t1{it}")
        nc.vector.tensor_mul(t1[:], y, y)                     # y^2
        t2 = psb.tile([1, S], F32, tag=f"{tag}_t2{it}", name=f"{tag}_t2{it}")
        # t2 = (y^2 * -0.5) * ss  (scalar_tensor_tensor)
        nc.vector.scalar_tensor_tensor(out=t2[:], in0=t1[:], scalar=-0.5,
                                       in1=ss[:], op0=mul, op1=mul)
        t3 = psb.tile([1, S], BF16 if it == 1 else F32,
                      tag=f"{tag}_t3{it}", name=f"{tag}_t3{it}")
        # y = y * (t2 + 1.5)
        nc.vector.scalar_tensor_tensor(out=t3[:], in0=t2[:], scalar=1.5,
                                       in1=y, op0=add, op1=mul)
        y = t3[:]
    return y


def _tower(nc, tc, pools, w, m, consts):
    """Project one tower and normalize.  Returns (zn fp32 [128,S],
    znb8 fp8 [128,2,S] with plane1 zeroed elsewhere)."""
    pps, psb, apool = pools
    add = mybir.AluOpType.add
    mx = mybir.AluOpType.max
    ones_col, ones_colb, ones_rowb = consts
    KO = DIN // 128

    h = psb.tile([128, (DE // 128) * S], BF16, tag="h")
    for mm in range(DE // 128):
        ph = pps.tile([128, S], F32, tag="ps")
        for k2 in range(0, KO, 2):
            nc.tensor.matmul(
                ph[:],
                w["w8"][:, :].rearrange("p (ko m) -> p ko m", ko=KO)
                    [:, k2:k2 + 2, 128 * mm:128 * (mm + 1)],
                w["x8"][:, :].rearrange("p (ko b) -> p ko b", ko=KO)
                    [:, k2:k2 + 2, :],
                start=(k2 == 0), stop=(k2 == KO - 2), perf_mode=DR)
        nc.scalar.activation(
            h[:, mm * S:(mm + 1) * S], ph[:],
            mybir.ActivationFunctionType.Identity,
            bias=w["beT"][:, mm:mm + 1], scale=1.0)
    g = psb.tile([128, (DH // 128) * S], BF16, tag="g")
    for mm in range(DH // 128):
        pg = pps.tile([128, S], F32, tag="ps")
        for k in range(DE // 128):
            nc.tensor.matmul(
                pg[:],
                w["wp1"][:, k * DH + 128 * mm: k * DH + 128 * (mm + 1)],
                h[:, k * S:(k + 1) * S],
                start=(k == 0), stop=(k == DE // 128 - 1))
        nc.scalar.activation(
            g[:, mm * S:(mm + 1) * S], pg[:],
            mybir.ActivationFunctionType.Relu,
            bias=w["bp1T"][:, mm:mm + 1], scale=1.0)
    pz = pps.tile([128, S], F32, tag="ps")
    for k in range(DH // 128):
        nc.tensor.matmul(pz[:], w["wp2"][:, k * DP: k * DP + 128],
                         g[:, k * S:(k + 1) * S],
                         start=(k == 0), stop=(k == DH // 128 - 1))
    z = psb.tile([128, S], F32, tag=f"z_{m}")
    nc.scalar.activation(z[:], pz[:], mybir.ActivationFunctionType.Identity,
                         bias=w["bp2T"][:, 0:1], scale=1.0)

    # ss = col sums of z^2 via bf16 ones-matmul (1 cyc/row)
    sq = psb.tile([128, S], BF16, tag="sq")
    nc.vector.tensor_mul(sq[:], z[:], z[:])
    pssq = pps.tile([1, S], F32, tag="ps")
    nc.tensor.matmul(pssq[:], ones_colb[:], sq[:], start=True, stop=True)
    inv = _rsqrt_dve(nc, psb, pssq, f"rs_{m}")
    pinvb = pps.tile([128, S], F32, tag="ps")
    nc.tensor.matmul(pinvb[:], ones_rowb[:], inv, start=True, stop=True)
    zn = apool.tile([128, S], F32, name=f"zn_{m}")
    nc.vector.tensor_mul(zn[:], z[:], pinvb[:])
    znb8 = apool.tile([128, 2, S], F8, name=f"znb8_{m}")
    nc.vector.tensor_copy(znb8[:, 0, :], zn[:])
    return zn, znb8


def _emit(nc, tc, t_in, out_t):
    Exp = mybir.ActivationFunctionType.Exp
    add = mybir.AluOpType.add
    mul = mybir.AluOpType.mult

    with tc.tile_pool(name="const", bufs=1) as cpool, \
         tc.tile_pool(name="wpool", bufs=1) as wpool, \
         tc.tile_pool(name="actpool", bufs=1) as apool, \
         tc.tile_pool(name="projsb", bufs=2) as psb, \
         tc.tile_pool(name="psum", bufs=2, space="PSUM") as pps, \
         tc.tile_pool(name="escp", bufs=2) as escp, \
         tc.tile_pool(name="dram", bufs=1, space="DRAM") as dram:

        ones_col = cpool.tile([128, 1], F32)
        nc.vector.memset(ones_col[:], 1.0)
        ones_colb = cpool.tile([128, 1], BF16)
        nc.vector.memset(ones_colb[:], 1.0)
        ones_rowb = cpool.tile([1, 128], BF16)
        nc.vector.memset(ones_rowb[:], 1.0)
        # Pin the ScalarE table to Exp once, at t=0.
        dummy = cpool.tile([1, 1], F32)
        nc.scalar.activation(dummy[:], ones_col[0:1, 0:1], Exp)

        # gathered projections [128, 2, 4096]; plane 1 stays zero (DR pad)
        zf8 = {m: apool.tile([128, 2, B], F8, name=f"zf8_{m}")
               for m in ("img", "txt")}
        for m in ("img", "txt"):
            nc.gpsimd.memset(zf8[m][:, 1, :], 0.0)

        # tower input DMAs: img on SP queue, txt on Pool queue (parallel)
        w_all = {"img": _load_tower(nc, wpool, t_in, "img", nc.sync),
                 "txt": _load_tower(nc, wpool, t_in, "txt", nc.gpsimd)}

        zn, znb8, cc_out = {}, {}, {}
        for m in ("img", "txt"):
            zn[m], znb8[m] = _tower(nc, tc, (pps, psb, apool), w_all[m], m,
                                    (ones_col, ones_colb, ones_rowb))
            nc.gpsimd.memset(znb8[m][:, 1, :], 0.0)
            # bounce fp8 plane-0 to DRAM on the ScalarE HWDGE queue
            cc_in = dram.tile([128, S], F8, name=f"cc_in_{m}")
            nc.scalar.dma_start(out=cc_in[:, :], in_=znb8[m][:, 0, :])
            cc_o = dram.tile([128 * NCORES, S], F8, name=f"cc_out_{m}",
                             addr_space="Shared")
            nc.gpsimd.collective_compute(
                "AllGather", mybir.AluOpType.bypass,
                replica_groups=[list(range(NCORES))],
                ins=[cc_in[:]], outs=[cc_o[:]])
            cc_out[m] = cc_o
            # gather loads ride the SP queue (idle after tower loads)
            for j in range(NCORES):
                nc.sync.dma_start(
                    out=zf8[m][:, 0, S * j: S * (j + 1)],
                    in_=cc_o[128 * j:128 * (j + 1), :])

        # pos / self-diag rows ([1, S] each) -> [128, 4] via DRAM scatter.
        # diag must be |z_fp8|^2 to cancel the sim matmul's self column.
        zq = {}
        for m in ("img", "txt"):
            zq[m] = psb.tile([128, S], BF16, tag=f"zq_{m}", name=f"zq_{m}")
            nc.vector.tensor_copy(zq[m][:], znb8[m][:, 0, :])
        rows_d = dram.tile([3, S], F32)
        for r, (a, b, src) in enumerate((("img", "txt", zn),
                                         ("img", "img", zq),
                                         ("txt", "txt", zq))):
            prod = psb.tile([128, S], F32, tag="prod")
            if src is zn:
                nc.vector.tensor_mul(prod[:], zn[a][:], zn[b][:])
            else:
                nc.vector.tensor_mul(prod[:], zq[a][:], zq[b][:])
            pr = pps.tile([1, S], F32, tag="ps")
            nc.tensor.matmul(pr[:], ones_col[:], prod[:], start=True,
                             stop=True)
            row_sb = psb.tile([1, S], F32, tag="rowsb")
            nc.vector.tensor_copy(row_sb[:], pr[:])
            nc.scalar.dma_start(out=rows_d[r:r + 1, :], in_=row_sb[:])
        pdT = apool.tile([128, 12], F32)
        for r in range(3):
            nc.scalar.dma_start(
                out=pdT[:, 4 * r:4 * (r + 1)],
                in_=rows_d[r:r + 1, :].rearrange("o (c p) -> (o p) c", p=128))

        # ---- sim units ----
        # unit (rc, uu): rows 128rc..128rc+128, cols [uu*2048, (uu+1)*2048)
        # uu 0..1 = img cols, 2..3 = txt cols.
        # ACT units: img units for rc<4, txt units for rc>=4 (diag lives
        # there for every core), + EXTRA_ACT txt units of rc<4 appended.
        # DVE units: the rest (Schraudolph).
        stats = apool.tile([128, NRC * NU], F32)

        def emit_unit(rc, uu, eng):
            m = "img" if uu < 2 else "txt"
            coff = (uu % 2) * UW
            lsrc = znb8["img"] if rc < 4 else znb8["txt"]
            lhs = lsrc[:, :, 128 * (rc % 4):128 * (rc % 4 + 1)]
            ps = pps.tile([128, UW], F32, tag="ps")
            for q in range(UW // 512):
                nc.tensor.matmul(
                    ps[:, 512 * q:512 * (q + 1)], lhs,
                    zf8[m][:, :, coff + 512 * q: coff + 512 * (q + 1)],
                    start=True, stop=True, perf_mode=DR)
            sc = stats[:, NU * rc + uu: NU * rc + uu + 1]
            if eng == "A":
                esc = escp.tile([128, UW], BF16, tag="esc")
                nc.scalar.activation(esc[:], ps[:], Exp, scale=INV_T,
                                     accum_out=sc)
            else:
                t16 = escp.tile([128, UW], I16, tag="t16")
                nc.vector.tensor_scalar(out=t16[:], in0=ps[:],
                                        scalar1=A1, scalar2=A2,
                                        op0=mul, op1=add)
                nc.vector.tensor_reduce(
                    out=sc, in_=t16[:].bitcast(BF16),
                    axis=mybir.AxisListType.X, op=add)

        units = []
        # img phase: ACT rc0-3, DVE rc4-7, interleaved for PSUM rotation
        for rc in range(4):
            for uu in range(2):
                units.append((rc, uu, "A"))
                units.append((rc + 4, uu, "D"))
        # txt phase: ACT rc4-7 (+extras from rc0-3), DVE rest of rc0-3
        extras = [(rc, uu) for rc in range(4) for uu in (2, 3)][:EXTRA_ACT]
        for rc in range(4):
            for uu in (2, 3):
                units.append((rc + 4, uu, "A"))
                units.append((rc, uu, "A" if (rc, uu) in extras else "D"))
        for rc, uu, eng in units:
            emit_unit(rc, uu, eng)

        # ---- gather outputs: T (8) | pos(4) | diag1(4) | diag2(4) ----
        outv = apool.tile([128, OUT_COLS], F32)
        nc.vector.tensor_reduce(
            out=outv[:, 0:NRC],
            in_=stats[:].rearrange("p (r u) -> p r u", u=NU),
            axis=mybir.AxisListType.X, op=add)
        nc.vector.tensor_copy(outv[:, NRC:NRC + 12], pdT[:])
        nc.sync.dma_start(out=out_t[:, :], in_=outv[:])


def _prep_in_maps(inputs):
    host = {}
    KO = DIN // 128
    for m in ("img", "txt"):
        We = np.asarray(inputs[f"We_{m}"], np.float32)     # [1024, 512]
        # [p, ko, m] = We[128*ko + p, m]
        host[f"We8_{m}"] = np.ascontiguousarray(
            We.reshape(KO, 128, DE).transpose(1, 0, 2).reshape(128, KO * DE)
        ).astype(NP_F8)
        host[f"Wp1_{m}"] = np.ascontiguousarray(inputs[f"Wp1_{m}"]).astype(NP_BF16)
        host[f"Wp2_{m}"] = np.ascontiguousarray(inputs[f"Wp2_{m}"]).astype(NP_BF16)
        host[f"beT_{m}"] = np.ascontiguousarray(
            np.asarray(inputs[f"be_{m}"], np.float32).reshape(DE // 128, 128).T)
        host[f"bp1T_{m}"] = np.ascontiguousarray(
            np.asarray(inputs[f"bp1_{m}"], np.float32).reshape(DH // 128, 128).T)
        host[f"bp2T_{m}"] = np.ascontiguousarray(
            np.asarray(inputs[f"bp2_{m}"], np.float32).reshape(DP // 128, 128).T)
    x = {"img": np.asarray(inputs["x_image"], np.float32),
         "txt": np.asarray(inputs["x_text"], np.float32)}
    in_maps = []
    for c in range(NCORES):
        mp = dict(host)
        for m in ("img", "txt"):
            xs = x[m][c * S:(c + 1) * S]                    # [512, 1024]
            # [p, ko, b] = x[b, 128*ko + p]
            mp[f"xT8_{m}"] = np.ascontiguousarray(
                xs.T.reshape(KO, 128, S).transpose(1, 0, 2).reshape(128, KO * S)
            ).astype(NP_F8)
        in_maps.append(mp)
    return in_maps


def _finish_host(results):
    """Host-side fp64 finish: combine per-core T/pos/diag into the loss."""
    total = 0.0
    t = TEMP
    for c in range(NCORES):
        p = np.asarray(results[c]["parts"], np.float64)
        T = p[:, 0:8]           # [128, rc]
        pos = p[:, 8:12]        # [128, k]  (k = batch chunk within shard)
        d1 = p[:, 12:16]
        d2 = p[:, 16:20]
        for rc in range(8):
            k = rc % 4
            dg = d1[:, k] if rc < 4 else d2[:, k]
            Tp = T[:, rc] - np.exp(dg / t) + np.exp(pos[:, k] / t)
            total += float(np.sum(np.log(Tp) - pos[:, k] / t))
    return np.float32(total / N)


def kernel(**inputs) -> np.ndarray:
    nc = _CACHE.get("nc")
    if nc is None:
        nc = _build()
        _CACHE["nc"] = nc
    res = run_bass_kernel_spmd(nc, _prep_in_maps(inputs),
                               core_ids=list(range(NCORES)))
    return _finish_host(res.results)
